# revision 41
# baseline (speedup 1.0000x reference)
"""Trainium2 Bass kernel for nn_ODEFunc_interaction (gnn_message_passing).

Math (see reference):
  dz_dt = tanh([z, t] @ vW1 + vb1) @ vW2 + vb2                    (v-net, all rows)
  for each pair (perm[2i], perm[2i+1]):
      d_i  = z[perm[2i]] - z[perm[2i+1]]
      g_i  = grad_phi(d_i) = pW1 @ (pW2[:,0] * (1 - tanh(d_i@pW1 + pb1)^2))
      out[perm[2i]]   = dz_dt[perm[2i]]   - g_i
      out[perm[2i+1]] = dz_dt[perm[2i+1]] + g_i
  last 3 rows (triple) handled on host in float64 (tiny).

Strategy: host gathers z[perm] so each of 8 cores owns a contiguous block of
200000/8 = 25000 rows (12500 pairs). On-device layout is transposed+packed:
X[128, 6250] where partition 32*j+d holds dim d of row-chunk j (4 chunks of
6250 rows). All matmuls stream fp16; tanh (+bias) on ACT; pair-diff and
square on GPSIMD/DVE; final +/- combine on DVE. Host scatters back by perm.
"""

import os
import numpy as np

B, D, H = 200003, 32, 128
NCORES = 8
P2 = 200000            # rows covered by pairs
RPC = P2 // NCORES     # 25000 rows per core
NCHUNK = 4
L = RPC // NCHUNK      # 6250 packed columns per core
LP = L + 2             # padded to keep every matmul free-size even
G = 1024               # column block (2 PSUM banks)

_CACHE = {}
LAST_RESULTS = None    # BassKernelResults of the most recent run (for test.py)


def build_program():
    """Build the single-core Bass/Tile program (same program runs SPMD on 8 cores)."""
    from contextlib import ExitStack
    import concourse.bacc as bacc
    import concourse.mybir as mybir
    import concourse.tile as tile

    dt = mybir.dt
    F32 = dt.float32
    AF = mybir.ActivationFunctionType
    OP = mybir.AluOpType

    F16 = dt.float16
    # All matmul streams run in fp16. One concatenated fp16 weight tensor
    # [128, 2048]:
    #   w1rep[0:128] | pw1rep[128:256] | w2q[256:768] | pwtq[768:1280]
    #   | pwtqn[1280:1792] | w1z[1792:1920] | pw1z[1920:2048]
    # w2q/pwtq are column-placed per chunk (vW2 at columns 32j of block j,
    # zeros elsewhere): matmul outputs must start at PSUM partition 0, so the
    # 4 chunk matmuls accumulate full-M into one [128,*] psum tile.
    # w1z/pw1z: chunk 3 is read from partition base 64 with K=64 and zeros in
    # rows 64:96 (partition base 96 is not encodable).
    nc = bacc.Bacc()
    X = nc.dram_tensor("x", [128, LP], F16, kind="ExternalInput")
    WC = nc.dram_tensor("wcat", [128, 2048], F16, kind="ExternalInput")
    BC = nc.dram_tensor("bias", [128, 2], F32, kind="ExternalInput")
    O = nc.dram_tensor("out", [128, LP], F32, kind="ExternalOutput")

    with tile.TileContext(nc) as tc, ExitStack() as ctx:
        wpool = ctx.enter_context(tc.tile_pool(name="wpool", bufs=1))
        xpool = ctx.enter_context(tc.tile_pool(name="xpool", bufs=5))
        upool = ctx.enter_context(tc.tile_pool(name="upool", bufs=5))
        vpool = ctx.enter_context(tc.tile_pool(name="vpool", bufs=3))
        sqpool = ctx.enter_context(tc.tile_pool(name="sqpool", bufs=4))
        dpool = ctx.enter_context(tc.tile_pool(name="dpool", bufs=3))
        qspool = ctx.enter_context(tc.tile_pool(name="qspool", bufs=3))
        opool = ctx.enter_context(tc.tile_pool(name="opool", bufs=4))
        hps = ctx.enter_context(tc.tile_pool(name="hps", bufs=2, space="PSUM"))
        dzps = ctx.enter_context(tc.tile_pool(name="dzps", bufs=1, space="PSUM"))
        aps = ctx.enter_context(tc.tile_pool(name="aps", bufs=1, space="PSUM"))
        qps = ctx.enter_context(tc.tile_pool(name="qps", bufs=1, space="PSUM"))

        wt = wpool.tile([128, 2048], F16)
        # split the weight DMA: the small bursts only need w1/pw1 (64KB);
        # the column-placed tail weights (448KB) load behind block 0's input
        # so the first matmul is not gated on the full 512KB transfer
        nc.sync.dma_start(wt[:, 0:256], WC[:, 0:256])
        bt = wpool.tile([128, 2], F32)
        nc.sync.dma_start(bt[:], BC[:])
        # warm the tanh ACT table during the startup DMA window so the first
        # real activation doesn't pay the ~1.3us table load on the critical path
        warm = wpool.tile([128, 1], F32)
        nc.scalar.activation(warm[:], bt[:, 0:1], AF.Tanh)
        w1 = wt[:, 0:128]
        pw1 = wt[:, 128:256]
        w2q = wt[:, 256:768]
        pwtq = wt[:, 768:1280]      # +pW1*w2 column-placed per chunk
        pwtqn = wt[:, 1280:1792]    # negated copy (odd output columns)
        w1z = wt[:, 1792:1920]
        pw1z = wt[:, 1920:2048]
        bh = bt[:, 0:1]
        pb1 = bt[:, 1:2]

        for c0 in range(0, LP, G):
            W_ = min(G, LP - c0)
            Wp = W_ // 2
            xt = xpool.tile([128, G], F16)
            nc.sync.dma_start(xt[:, :W_], X[:, c0 : c0 + W_])
            if c0 == 0:
                nc.sync.dma_start(wt[:, 256:1280], WC[:, 256:1280])

            df = dpool.tile([128, G // 2], F16)
            nc.vector.tensor_tensor(df[:, :Wp], xt[:, 0:W_:2], xt[:, 1:W_:2], OP.subtract)

            dz = dzps.tile([128, G], F32)
            qp = qps.tile([128, G // 2], F32)
            ot = opool.tile([128, G], F32)

            # Emission order is tuned for the PE's in-order queue:
            # - matmuls are batched by PE tile config (all (32,128)-tiles
            #   first at rotating positions 96/0/96/0/32/64, then all K=128
            #   full-array), because config switches cost ~400ns and kill
            #   back-to-back overlap;
            # - consecutive small-tile matmuls never share a tile position
            #   and consecutive K=128 matmuls never accumulate into the same
            #   psum bank (both serialize otherwise);
            # - pa psum lives in the dz/qp/aps banks (consumed by the tanh
            #   before the tail burst overwrites them), fitting 8 banks.
            def mm_h(j, s0, ph):
                sw = min(512, W_ - s0)
                if sw <= 0:
                    return
                p0 = 32 * j
                nc.tensor.matmul(
                    ph[:, s0 : s0 + sw],
                    w1[p0 : p0 + 32, :],
                    xt[p0 : p0 + 32, s0 : s0 + sw],
                    start=True,
                    stop=True,
                    tile_position=(p0, 0),
                )

            def mm_pa(j, pa):
                p0 = 32 * j
                nc.tensor.matmul(
                    pa[:, :Wp],
                    pw1[p0 : p0 + 32, :],
                    df[p0 : p0 + 32, :Wp],
                    start=True,
                    stop=True,
                    tile_position=(p0, 0),
                    skip_group_check=True,
                )

            def act_h(j, ph):
                ut = upool.tile([128, G], F16, name="ut")
                nc.scalar.activation(ut[:, :W_], ph[:, :W_], AF.Tanh, bias=bh[:])
                return ut

            def act_pa(j, pa):
                vt = vpool.tile([128, G // 2], F16, name="vt")
                nc.scalar.activation(vt[:, :Wp], pa[:, :Wp], AF.Tanh, bias=pb1[:])
                sq = sqpool.tile([128, G // 2], F16, name="sq")
                nc.vector.tensor_mul(sq[:, :Wp], vt[:, :Wp], vt[:, :Wp])
                return sq

            def mm_dz(j, s0, ut, start, stop):
                sw = min(512, W_ - s0)
                if sw <= 0:
                    return
                nc.tensor.matmul(
                    dz[:, s0 : s0 + sw],
                    w2q[:, H * j : H * (j + 1)],
                    ut[:, s0 : s0 + sw],
                    start=start,
                    stop=stop,
                    skip_group_check=True,
                )

            def mm_qp(j, sq, start, stop):
                nc.tensor.matmul(
                    qp[:, :Wp],
                    pwtq[:, H * j : H * (j + 1)],
                    sq[:, :Wp],
                    start=start,
                    stop=stop,
                    skip_group_check=True,
                )

            pav = aps.tile([128, G // 2], F32)
            pa_reg = {
                3: pav,                    # first pa: no cross-block wait
                0: dz[:, 0 : G // 2],      # dz/qp free once prev combine read
                1: dz[:, G // 2 : G],
                2: qp[:, 0 : G // 2],
            }

            ph3 = hps.tile([128, G], F32, name="ph")
            ph0 = hps.tile([128, G], F32, name="ph")
            mm_h(3, 0, ph3)
            mm_h(0, 0, ph0)
            mm_h(3, 512, ph3)
            ut3 = act_h(3, ph3)
            mm_h(0, 512, ph0)
            ut0 = act_h(0, ph0)
            mm_pa(3, pa_reg[3])
            sq3 = act_pa(3, pa_reg[3])
            mm_pa(0, pa_reg[0])
            ph1 = hps.tile([128, G], F32, name="ph")
            ph2 = hps.tile([128, G], F32, name="ph")
            mm_h(1, 0, ph1)
            mm_h(2, 0, ph2)
            mm_h(1, 512, ph1)
            ut1 = act_h(1, ph1)
            mm_h(2, 512, ph2)
            ut2 = act_h(2, ph2)
            mm_pa(1, pa_reg[1])
            if W_ == G:
                # pa0/pa1 contiguous in dz[:, 0:G]: one merged tanh + mul
                vt01 = vpool.tile([128, G], F16, name="vt01")
                nc.scalar.activation(vt01[:], dz[:], AF.Tanh, bias=pb1[:])
                sq01 = sqpool.tile([128, G], F16, name="sq01")
                nc.vector.tensor_mul(sq01[:], vt01[:], vt01[:])
                sq0 = sq01[:, 0 : G // 2]
                sq1 = sq01[:, G // 2 : G]
            else:
                sq0 = act_pa(0, pa_reg[0])
                sq1 = act_pa(1, pa_reg[1])
            mm_pa(2, pa_reg[2])
            sq2 = act_pa(2, pa_reg[2])

            uts = {3: ut3, 0: ut0, 1: ut1, 2: ut2}
            sqs = {3: sq3, 0: sq0, 1: sq1, 2: sq2}
            # tail burst: uniform K=128 config, bank-interleaved.
            # start/stop are per psum region: each 512-col dz region has its
            # own j3..j2 accumulation group, as does qp.
            mm_dz(3, 0, uts[3], True, False)
            mm_qp(3, sqs[3], True, False)
            mm_dz(3, 512, uts[3], True, False)
            mm_dz(0, 0, uts[0], False, False)
            mm_qp(0, sqs[0], False, False)
            mm_dz(0, 512, uts[0], False, False)
            mm_dz(1, 0, uts[1], False, False)
            mm_qp(1, sqs[1], False, False)
            mm_dz(1, 512, uts[1], False, False)
            mm_dz(2, 0, uts[2], False, True)
            mm_qp(2, sqs[2], False, True)
            mm_dz(2, 512, uts[2], False, True)

            qs = qspool.tile([128, G // 2], F32)
            nc.vector.tensor_copy(qs[:, :Wp], qp[:, :Wp])
            nc.vector.tensor_tensor(ot[:, 0:W_:2], dz[:, 0:W_:2], qs[:, :Wp], OP.add)
            nc.vector.tensor_tensor(ot[:, 1:W_:2], dz[:, 1:W_:2], qs[:, :Wp], OP.subtract)
            nc.sync.dma_start(O[:, c0 : c0 + W_], ot[:, :W_])

    nc.compile()
    return nc


def _prep_weights(t, vW1, vb1, vW2, vb2, pW1, pb1, pW2):
    f32 = np.float32
    t = np.asarray(t, dtype=f32).reshape(-1)[0]
    vW1 = np.asarray(vW1, dtype=f32)
    w1rep = np.tile(np.ascontiguousarray(vW1[:32]), (4, 1))            # [128,128]
    biash = (np.asarray(vb1, f32) + t * vW1[32]).reshape(128, 1).astype(f32)
    vw2 = np.ascontiguousarray(np.asarray(vW2, f32))                   # [128,32]
    pW1 = np.asarray(pW1, f32)
    pw1rep = np.tile(pW1, (4, 1))                                      # [128,128]
    pb1c = np.asarray(pb1, f32).reshape(128, 1).copy()
    w2col = np.asarray(pW2, f32).reshape(128)
    pw1tw2 = np.ascontiguousarray((pW1 * w2col[None, :]).T)            # [128,32]
    z96 = np.zeros((96, 128), f32)
    w2q = np.zeros((128, 512), f32)
    pwtq = np.zeros((128, 512), f32)
    for j in range(4):
        w2q[:, 128 * j + 32 * j : 128 * j + 32 * j + 32] = vw2
        pwtq[:, 128 * j + 32 * j : 128 * j + 32 * j + 32] = pw1tw2
    w1z = np.vstack([z96, vW1[:32]])                                   # [128,128]
    pw1z = np.vstack([z96, pW1])                                       # [128,128]
    wcat = np.hstack([w1rep, pw1rep, w2q, pwtq, -pwtq, w1z, pw1z]).astype(np.float16)
    bias = np.hstack([biash, pb1c]).astype(f32)
    # constant part of g: c0[d] = sum_k pW1[d,k]*w2[k], in the fp16 weight
    # precision actually used on device
    c0base = pw1tw2.astype(np.float16).astype(f32).sum(axis=0)         # [32]
    return {"wcat": np.ascontiguousarray(wcat), "bias": np.ascontiguousarray(bias),
            "_c0base": c0base}


def _pack_core(zc):
    """[25000, 32] f32 -> [128, 6252] fp16 packed (partition 32*j+d, col i =
    row j*L+i), padded with 2 zero columns."""
    out = np.zeros((128, LP), dtype=np.float16)
    out[:, :L] = zc.reshape(NCHUNK, L, 32).transpose(0, 2, 1).reshape(128, L)
    return out


def _unpack_core(oc):
    """[128, 6252] packed -> [25000, 32]."""
    return oc[:, :L].reshape(NCHUNK, 32, L).transpose(0, 2, 1).reshape(RPC, 32)


def _host_triple(t, z3, vW1, vb1, vW2, vb2, pW1, pb1, pW2):
    """Exact float64 computation of the 3 leftover rows: dz_dt + triple forces."""
    f8 = np.float64
    z3 = z3.astype(f8)
    vW1 = np.asarray(vW1, f8)
    t = float(np.asarray(t).reshape(-1)[0])
    h3 = np.tanh(z3 @ vW1[:32] + t * vW1[32] + np.asarray(vb1, f8))
    dz3 = h3 @ np.asarray(vW2, f8) + np.asarray(vb2, f8)

    pW1 = np.asarray(pW1, f8)
    w2 = np.asarray(pW2, f8).reshape(128)
    d9 = (z3[:, None, :] - z3[None, :, :]).reshape(9, 32)
    u9 = np.tanh(d9 @ pW1 + np.asarray(pb1, f8))
    s9 = (1.0 - u9 * u9) * w2[None, :]
    g9 = s9 @ pW1.T                       # grad_phi rows
    f9 = (-g9).reshape(3, 3, 32)
    f9 = f9 * (1.0 - np.eye(3)[:, :, None])
    force3 = f9.sum(axis=1) * 2.0
    return (dz3 + force3).astype(np.float32)


def kernel(t, z, perm, vW1, vb1, vW2, vb2, pW1, pb1, pW2, pb2):
    from concourse.bass_utils import run_bass_kernel_spmd

    global LAST_RESULTS
    if "nc" not in _CACHE:
        _CACHE["nc"] = build_program()
    nc = _CACHE["nc"]

    z = np.asarray(z, np.float32)
    perm = np.asarray(perm)
    weights = _prep_weights(t, vW1, vb1, vW2, vb2, pW1, pb1, pW2)

    c0base = weights.pop("_c0base")
    zg = z[perm[:P2]]                       # [200000, 32] gathered pair rows
    in_maps = []
    for c in range(NCORES):
        im = {"x": _pack_core(zg[c * RPC : (c + 1) * RPC])}
        im.update(weights)
        in_maps.append(im)

    trace = bool(int(os.environ.get("KERNEL_TRACE", "0")))
    res = run_bass_kernel_spmd(nc, in_maps, list(range(NCORES)), trace=trace)
    LAST_RESULTS = res

    out = np.empty((B, 32), dtype=np.float32)
    og = np.concatenate([_unpack_core(res.results[c]["out"]) for c in range(NCORES)], axis=0)
    vb2f = np.asarray(vb2, np.float32)
    og[0::2] += (vb2f - c0base)[None, :]
    og[1::2] += (vb2f + c0base)[None, :]
    out[perm[:P2]] = og
    out[perm[P2:]] = _host_triple(t, z[perm[P2:]], vW1, vb1, vW2, vb2, pW1, pb1, pW2)
    return out


# revision 42
# speedup vs baseline: 1.0217x; 1.0217x over previous
"""Trainium2 Bass kernel for nn_ODEFunc_interaction (gnn_message_passing).

Math (see reference):
  dz_dt = tanh([z, t] @ vW1 + vb1) @ vW2 + vb2                    (v-net, all rows)
  for each pair (perm[2i], perm[2i+1]):
      d_i  = z[perm[2i]] - z[perm[2i+1]]
      g_i  = grad_phi(d_i) = pW1 @ (pW2[:,0] * (1 - tanh(d_i@pW1 + pb1)^2))
      out[perm[2i]]   = dz_dt[perm[2i]]   - g_i
      out[perm[2i+1]] = dz_dt[perm[2i+1]] + g_i
  last 3 rows (triple) handled on host in float64 (tiny).

Strategy: host gathers z[perm] so each of 8 cores owns a contiguous block of
200000/8 = 25000 rows (12500 pairs). On-device layout is transposed+packed:
X[128, 6250] where partition 32*j+d holds dim d of row-chunk j (4 chunks of
6250 rows). All matmuls stream fp16; tanh (+bias) on ACT; pair-diff and
square on GPSIMD/DVE; final +/- combine on DVE. Host scatters back by perm.
"""

import os
import numpy as np

B, D, H = 200003, 32, 128
NCORES = 8
P2 = 200000            # rows covered by pairs
RPC = P2 // NCORES     # 25000 rows per core
NCHUNK = 4
L = RPC // NCHUNK      # 6250 packed columns per core
LP = L + 2             # padded to keep every matmul free-size even
G = 1024               # column block (2 PSUM banks)

_CACHE = {}
LAST_RESULTS = None    # BassKernelResults of the most recent run (for test.py)


def build_program():
    """Build the single-core Bass/Tile program (same program runs SPMD on 8 cores)."""
    from contextlib import ExitStack
    import concourse.bacc as bacc
    import concourse.mybir as mybir
    import concourse.tile as tile

    dt = mybir.dt
    F32 = dt.float32
    AF = mybir.ActivationFunctionType
    OP = mybir.AluOpType

    F16 = dt.float16
    # All matmul streams run in fp16. One concatenated fp16 weight tensor
    # [128, 2048]:
    #   w1rep[0:128] | pw1rep[128:256] | w2q[256:768] | pwtq[768:1280]
    #   | pwtqn[1280:1792] | w1z[1792:1920] | pw1z[1920:2048]
    # w2q/pwtq are column-placed per chunk (vW2 at columns 32j of block j,
    # zeros elsewhere): matmul outputs must start at PSUM partition 0, so the
    # 4 chunk matmuls accumulate full-M into one [128,*] psum tile.
    # w1z/pw1z: chunk 3 is read from partition base 64 with K=64 and zeros in
    # rows 64:96 (partition base 96 is not encodable).
    nc = bacc.Bacc()
    X = nc.dram_tensor("x", [128, LP], F16, kind="ExternalInput")
    WC = nc.dram_tensor("wcat", [128, 2048], F16, kind="ExternalInput")
    BC = nc.dram_tensor("bias", [128, 2], F32, kind="ExternalInput")
    O = nc.dram_tensor("out", [128, LP], F32, kind="ExternalOutput")

    with tile.TileContext(nc) as tc, ExitStack() as ctx:
        wpool = ctx.enter_context(tc.tile_pool(name="wpool", bufs=1))
        xpool = ctx.enter_context(tc.tile_pool(name="xpool", bufs=4))
        upool = ctx.enter_context(tc.tile_pool(name="upool", bufs=5))
        vpool = ctx.enter_context(tc.tile_pool(name="vpool", bufs=3))
        sqpool = ctx.enter_context(tc.tile_pool(name="sqpool", bufs=4))
        dpool = ctx.enter_context(tc.tile_pool(name="dpool", bufs=2))
        qspool = ctx.enter_context(tc.tile_pool(name="qspool", bufs=2))
        opool = ctx.enter_context(tc.tile_pool(name="opool", bufs=3))
        hps = ctx.enter_context(tc.tile_pool(name="hps", bufs=2, space="PSUM"))
        dzps = ctx.enter_context(tc.tile_pool(name="dzps", bufs=1, space="PSUM"))
        aps = ctx.enter_context(tc.tile_pool(name="aps", bufs=1, space="PSUM"))
        qps = ctx.enter_context(tc.tile_pool(name="qps", bufs=1, space="PSUM"))

        wt = wpool.tile([128, 2048], F16)
        # split the weight DMA: the small bursts only need w1/pw1 (64KB);
        # the column-placed tail weights (448KB) load behind block 0's input
        # so the first matmul is not gated on the full 512KB transfer
        nc.sync.dma_start(wt[:, 0:256], WC[:, 0:256])
        bt = wpool.tile([128, 2], F32)
        nc.sync.dma_start(bt[:], BC[:])
        # warm the tanh ACT table during the startup DMA window so the first
        # real activation doesn't pay the ~1.3us table load on the critical path
        warm = wpool.tile([128, 1], F32)
        nc.scalar.activation(warm[:], bt[:, 0:1], AF.Tanh)
        w1 = wt[:, 0:128]
        pw1 = wt[:, 128:256]
        w2q = wt[:, 256:768]
        pwtq = wt[:, 768:1280]      # +pW1*w2 column-placed per chunk
        pwtqn = wt[:, 1280:1792]    # negated copy (odd output columns)
        w1z = wt[:, 1792:1920]
        pw1z = wt[:, 1920:2048]
        bh = bt[:, 0:1]
        pb1 = bt[:, 1:2]

        for c0 in range(0, LP, G):
            W_ = min(G, LP - c0)
            Wp = W_ // 2
            xt = xpool.tile([128, G], F16)
            nc.sync.dma_start(xt[:, :W_], X[:, c0 : c0 + W_])
            if c0 == 0:
                nc.sync.dma_start(wt[:, 256:1280], WC[:, 256:1280])

            df = dpool.tile([128, G // 2], F16)
            nc.vector.tensor_tensor(df[:, :Wp], xt[:, 0:W_:2], xt[:, 1:W_:2], OP.subtract)

            dz = dzps.tile([128, G], F32)
            qp = qps.tile([128, G // 2], F32)
            ot = opool.tile([128, G], F32)

            # Emission order is tuned for the PE's in-order queue:
            # - matmuls are batched by PE tile config (all (32,128)-tiles
            #   first at rotating positions 96/0/96/0/32/64, then all K=128
            #   full-array), because config switches cost ~400ns and kill
            #   back-to-back overlap;
            # - consecutive small-tile matmuls never share a tile position
            #   and consecutive K=128 matmuls never accumulate into the same
            #   psum bank (both serialize otherwise);
            # - pa psum lives in the dz/qp/aps banks (consumed by the tanh
            #   before the tail burst overwrites them), fitting 8 banks.
            def mm_h(j, s0, ph):
                sw = min(512, W_ - s0)
                if sw <= 0:
                    return
                p0 = 32 * j
                nc.tensor.matmul(
                    ph[:, s0 : s0 + sw],
                    w1[p0 : p0 + 32, :],
                    xt[p0 : p0 + 32, s0 : s0 + sw],
                    start=True,
                    stop=True,
                    tile_position=(p0, 0),
                )

            def mm_pa(j, pa):
                p0 = 32 * j
                nc.tensor.matmul(
                    pa[:, :Wp],
                    pw1[p0 : p0 + 32, :],
                    df[p0 : p0 + 32, :Wp],
                    start=True,
                    stop=True,
                    tile_position=(p0, 0),
                    skip_group_check=True,
                )

            def act_h(j, ph):
                ut = upool.tile([128, G], F16, name="ut")
                nc.scalar.activation(ut[:, :W_], ph[:, :W_], AF.Tanh, bias=bh[:])
                return ut

            def act_pa(j, pa):
                vt = vpool.tile([128, G // 2], F16, name="vt")
                nc.scalar.activation(vt[:, :Wp], pa[:, :Wp], AF.Tanh, bias=pb1[:])
                sq = sqpool.tile([128, G // 2], F16, name="sq")
                nc.vector.tensor_mul(sq[:, :Wp], vt[:, :Wp], vt[:, :Wp])
                return sq

            def mm_dz(j, s0, ut, start, stop):
                sw = min(512, W_ - s0)
                if sw <= 0:
                    return
                nc.tensor.matmul(
                    dz[:, s0 : s0 + sw],
                    w2q[:, H * j : H * (j + 1)],
                    ut[:, s0 : s0 + sw],
                    start=start,
                    stop=stop,
                    skip_group_check=True,
                )

            def mm_qp(j, sq, start, stop):
                nc.tensor.matmul(
                    qp[:, :Wp],
                    pwtq[:, H * j : H * (j + 1)],
                    sq[:, :Wp],
                    start=start,
                    stop=stop,
                    skip_group_check=True,
                )

            pav = aps.tile([128, G // 2], F32)
            pa_reg = {
                3: pav,                    # first pa: no cross-block wait
                0: dz[:, 0 : G // 2],      # dz/qp free once prev combine read
                1: dz[:, G // 2 : G],
                2: qp[:, 0 : G // 2],
            }

            ph3 = hps.tile([128, G], F32, name="ph")
            ph0 = hps.tile([128, G], F32, name="ph")
            mm_h(3, 0, ph3)
            mm_h(0, 0, ph0)
            mm_h(3, 512, ph3)
            ut3 = act_h(3, ph3)
            mm_h(0, 512, ph0)
            ut0 = act_h(0, ph0)
            mm_pa(3, pa_reg[3])
            sq3 = act_pa(3, pa_reg[3])
            mm_pa(0, pa_reg[0])
            ph1 = hps.tile([128, G], F32, name="ph")
            ph2 = hps.tile([128, G], F32, name="ph")
            mm_h(1, 0, ph1)
            mm_h(2, 0, ph2)
            mm_h(1, 512, ph1)
            ut1 = act_h(1, ph1)
            mm_h(2, 512, ph2)
            ut2 = act_h(2, ph2)
            mm_pa(1, pa_reg[1])
            if W_ == G:
                # pa0/pa1 contiguous in dz[:, 0:G]: one merged tanh + mul
                vt01 = vpool.tile([128, G], F16, name="vt01")
                nc.scalar.activation(vt01[:], dz[:], AF.Tanh, bias=pb1[:])
                sq01 = sqpool.tile([128, G], F16, name="sq01")
                nc.vector.tensor_mul(sq01[:], vt01[:], vt01[:])
                sq0 = sq01[:, 0 : G // 2]
                sq1 = sq01[:, G // 2 : G]
            else:
                sq0 = act_pa(0, pa_reg[0])
                sq1 = act_pa(1, pa_reg[1])
            mm_pa(2, pa_reg[2])
            sq2 = act_pa(2, pa_reg[2])

            uts = {3: ut3, 0: ut0, 1: ut1, 2: ut2}
            sqs = {3: sq3, 0: sq0, 1: sq1, 2: sq2}
            # tail burst: uniform K=128 config, bank-interleaved.
            # start/stop are per psum region: each 512-col dz region has its
            # own j3..j2 accumulation group, as does qp.
            mm_dz(3, 0, uts[3], True, False)
            mm_qp(3, sqs[3], True, False)
            mm_dz(3, 512, uts[3], True, False)
            mm_dz(0, 0, uts[0], False, False)
            mm_qp(0, sqs[0], False, False)
            mm_dz(0, 512, uts[0], False, False)
            mm_dz(1, 0, uts[1], False, False)
            mm_qp(1, sqs[1], False, False)
            mm_dz(1, 512, uts[1], False, False)
            mm_dz(2, 0, uts[2], False, True)
            mm_qp(2, sqs[2], False, True)
            mm_dz(2, 512, uts[2], False, True)

            qs = qspool.tile([128, G // 2], F32)
            nc.vector.tensor_copy(qs[:, :Wp], qp[:, :Wp])
            nc.vector.tensor_tensor(ot[:, 0:W_:2], dz[:, 0:W_:2], qs[:, :Wp], OP.add)
            nc.vector.tensor_tensor(ot[:, 1:W_:2], dz[:, 1:W_:2], qs[:, :Wp], OP.subtract)
            nc.sync.dma_start(O[:, c0 : c0 + W_], ot[:, :W_])

    nc.compile()
    return nc


def _prep_weights(t, vW1, vb1, vW2, vb2, pW1, pb1, pW2):
    f32 = np.float32
    t = np.asarray(t, dtype=f32).reshape(-1)[0]
    vW1 = np.asarray(vW1, dtype=f32)
    w1rep = np.tile(np.ascontiguousarray(vW1[:32]), (4, 1))            # [128,128]
    biash = (np.asarray(vb1, f32) + t * vW1[32]).reshape(128, 1).astype(f32)
    vw2 = np.ascontiguousarray(np.asarray(vW2, f32))                   # [128,32]
    pW1 = np.asarray(pW1, f32)
    pw1rep = np.tile(pW1, (4, 1))                                      # [128,128]
    pb1c = np.asarray(pb1, f32).reshape(128, 1).copy()
    w2col = np.asarray(pW2, f32).reshape(128)
    pw1tw2 = np.ascontiguousarray((pW1 * w2col[None, :]).T)            # [128,32]
    z96 = np.zeros((96, 128), f32)
    w2q = np.zeros((128, 512), f32)
    pwtq = np.zeros((128, 512), f32)
    for j in range(4):
        w2q[:, 128 * j + 32 * j : 128 * j + 32 * j + 32] = vw2
        pwtq[:, 128 * j + 32 * j : 128 * j + 32 * j + 32] = pw1tw2
    w1z = np.vstack([z96, vW1[:32]])                                   # [128,128]
    pw1z = np.vstack([z96, pW1])                                       # [128,128]
    wcat = np.hstack([w1rep, pw1rep, w2q, pwtq, -pwtq, w1z, pw1z]).astype(np.float16)
    bias = np.hstack([biash, pb1c]).astype(f32)
    # constant part of g: c0[d] = sum_k pW1[d,k]*w2[k], in the fp16 weight
    # precision actually used on device
    c0base = pw1tw2.astype(np.float16).astype(f32).sum(axis=0)         # [32]
    return {"wcat": np.ascontiguousarray(wcat), "bias": np.ascontiguousarray(bias),
            "_c0base": c0base}


def _pack_core(zc):
    """[25000, 32] f32 -> [128, 6252] fp16 packed (partition 32*j+d, col i =
    row j*L+i), padded with 2 zero columns."""
    out = np.zeros((128, LP), dtype=np.float16)
    out[:, :L] = zc.reshape(NCHUNK, L, 32).transpose(0, 2, 1).reshape(128, L)
    return out


def _unpack_core(oc):
    """[128, 6252] packed -> [25000, 32]."""
    return oc[:, :L].reshape(NCHUNK, 32, L).transpose(0, 2, 1).reshape(RPC, 32)


def _host_triple(t, z3, vW1, vb1, vW2, vb2, pW1, pb1, pW2):
    """Exact float64 computation of the 3 leftover rows: dz_dt + triple forces."""
    f8 = np.float64
    z3 = z3.astype(f8)
    vW1 = np.asarray(vW1, f8)
    t = float(np.asarray(t).reshape(-1)[0])
    h3 = np.tanh(z3 @ vW1[:32] + t * vW1[32] + np.asarray(vb1, f8))
    dz3 = h3 @ np.asarray(vW2, f8) + np.asarray(vb2, f8)

    pW1 = np.asarray(pW1, f8)
    w2 = np.asarray(pW2, f8).reshape(128)
    d9 = (z3[:, None, :] - z3[None, :, :]).reshape(9, 32)
    u9 = np.tanh(d9 @ pW1 + np.asarray(pb1, f8))
    s9 = (1.0 - u9 * u9) * w2[None, :]
    g9 = s9 @ pW1.T                       # grad_phi rows
    f9 = (-g9).reshape(3, 3, 32)
    f9 = f9 * (1.0 - np.eye(3)[:, :, None])
    force3 = f9.sum(axis=1) * 2.0
    return (dz3 + force3).astype(np.float32)


def kernel(t, z, perm, vW1, vb1, vW2, vb2, pW1, pb1, pW2, pb2):
    from concourse.bass_utils import run_bass_kernel_spmd

    global LAST_RESULTS
    if "nc" not in _CACHE:
        _CACHE["nc"] = build_program()
    nc = _CACHE["nc"]

    z = np.asarray(z, np.float32)
    perm = np.asarray(perm)
    weights = _prep_weights(t, vW1, vb1, vW2, vb2, pW1, pb1, pW2)

    c0base = weights.pop("_c0base")
    zg = z[perm[:P2]]                       # [200000, 32] gathered pair rows
    in_maps = []
    for c in range(NCORES):
        im = {"x": _pack_core(zg[c * RPC : (c + 1) * RPC])}
        im.update(weights)
        in_maps.append(im)

    trace = bool(int(os.environ.get("KERNEL_TRACE", "0")))
    res = run_bass_kernel_spmd(nc, in_maps, list(range(NCORES)), trace=trace)
    LAST_RESULTS = res

    out = np.empty((B, 32), dtype=np.float32)
    og = np.concatenate([_unpack_core(res.results[c]["out"]) for c in range(NCORES)], axis=0)
    vb2f = np.asarray(vb2, np.float32)
    og[0::2] += (vb2f - c0base)[None, :]
    og[1::2] += (vb2f + c0base)[None, :]
    out[perm[:P2]] = og
    out[perm[P2:]] = _host_triple(t, z[perm[P2:]], vW1, vb1, vW2, vb2, pW1, pb1, pW2)
    return out
